# revision 49
# baseline (speedup 1.0000x reference)
"""GAU (Gated Attention Unit) fused kernel for Trainium2, SPMD over 8 NeuronCores.

Sharding: data-parallel over batch (B=4) x query-sequence-halves (2) = 8 cores.
Each core computes the full GAU for its (batch, query-half).  The key sequence
is ROTATED host-side (np.roll by -off) so the query rows are always rows
[0:SQ) of xk — the query-side LayerNorm/transpose/qk-projection are then
slices of the key-side work (attention is permutation-invariant over j; the
T5 bias table is built for the rotated ordering, still Toeplitz in j-i).

Engine budget choices (vs the naive version):
  - ln_g/ln_b folded into the projection weights host-side; the PE-transpose
    evacuation is a pure copy, split between Act and DVE.
  - All free-dim bias adds (bhv, T5 bias, bo) are PSUM-accumulated on the PE
    via K=1 ones-matmuls / identity-matmuls (matmul cost is independent of K),
    removing ~290 [128,512] DVE ops.
  - v lives entirely in SBUF ([128, 32, 2048] fp8 = 64KB/partition): no DRAM
    roundtrip and no per-i-block re-read.
  - DMA descriptors are issued from SP/Pool, keeping Act/DVE clean.

Layout (all big matmuls fp8 inputs, DoubleRow, fp32 PSUM):
  - normed^T [d-part, s] via PE transpose; lhsT for v-proj, rhs for k/q/gate.
  - k^T/q^T [qk-dim, s]; sim^T[j,i] via one bf16 matmul (K=qk=128) + ident@bt
    bias accumulate; attn2 = relu(s)*s in one DVE op straight from PSUM.
  - out[i,h] accumulated with lhsT=attn2 slices, rhs=v tiles from SBUF.
  - gate multiply into goT [h,i]; final projection lhsT=goT, rhs=Wo, with
    bo/descale ones-matmul; epilogue = one DVE op (ps*descale + x).
"""

import math
import os
import sys

for _p in ("/opt/trn_rl_repo", "/root/.axon_site/_ro/trn_rl_repo"):
    if os.path.isdir(_p) and _p not in sys.path:
        sys.path.append(_p)

import numpy as np
import ml_dtypes

import concourse.bass as bass
import concourse.tile as tile
from concourse import mybir
from concourse.bass_utils import run_bass_kernel_spmd
from concourse.masks import make_identity

# Problem dims (hardcoded per spec)
B, S, D, QK, H = 4, 4096, 1024, 128, 2048
NUM_BUCKETS, MAX_DIST = 32, 128
LN_EPS = 1e-5
N_CORES = 8

P = 128
NB = 512  # free-dim block for matmuls

BF16 = mybir.dt.bfloat16
FP8 = mybir.dt.float8e4
F32 = mybir.dt.float32
ATTN_PRESCALE = 1024.0          # folded into g0/b0/bt host-side
ATTN_DESCALE = 1.0 / (ATTN_PRESCALE * ATTN_PRESCALE)

_NC_CACHE = {}


def _split_excess_waits(nc, max_waits=1):
    """This container's walrus rejects instructions carrying more than one
    sem wait ("Too many sync wait commands").  Move excess waits onto
    same-engine nops inserted immediately before the instruction — engine
    FIFO order makes that semantically identical."""
    f = nc.m.functions[0]
    for bb in list(f.blocks):
        il = list(bb.instructions)
        out = []
        changed = False
        for inst in il:
            si = inst.sync_info
            if si is not None and si.on_wait and len(si.on_wait) > max_waits:
                waits = list(si.on_wait)
                moved, keep = waits[:-max_waits], waits[-max_waits:]
                si.on_wait = keep
                for w in moved:
                    eng = nc.engines[inst.engine]
                    cur_bb = nc.cur_bb.bb
                    n_before = len(cur_bb.instructions)
                    nop = eng.nop()
                    # pop the freshly appended nop from wherever it landed
                    tail = list(cur_bb.instructions)
                    assert tail[-1] is nop.ins and len(tail) == n_before + 1
                    cur_bb.instructions = tail[:-1]
                    nsi = nop.ins.sync_info
                    if nsi is None:
                        nop.ins.sync_info = mybir.SyncInfo(
                            on_wait=[w], on_update=[])
                    else:
                        nsi.on_wait = [w]
                    out.append(nop.ins)
                changed = True
            out.append(inst)
        if changed:
            bb.instructions = out


def _install_drain_wait_split():
    """The walrus build in this container rejects >1 sem wait on the Tile
    epilogue Drain ("Too many sync wait commands").  Split the extra waits
    onto explicit SP nops (they only need to precede the final barrier)."""
    from concourse.vector_clock import ScopedClock

    if getattr(tile.TileContext, "_drain_split_installed", False):
        return

    def _patched(self, tick_clock, wait_clock):
        drain_inst = self.nc.sync.drain()
        wait_clock.add_sem_waits(
            drain_inst.ins, ScopedClock({None: tick_clock.global_clock}))
        si = drain_inst.ins.sync_info
        if si is not None and si.on_wait and len(si.on_wait) > 1:
            extra = list(si.on_wait)[1:]
            si.on_wait = [si.on_wait[0]]
            for w in extra:
                nop = self.nc.sync.nop()
                nsi = nop.ins.sync_info
                if nsi is None:
                    nop.ins.sync_info = mybir.SyncInfo(on_wait=[w], on_update=[])
                else:
                    nsi.on_wait = [w]
        self.nc.all_engine_barrier()
        assert self.sems is not None
        popped = self.nc._tile_sem_poison_stack.pop()
        assert popped is self._sem_poison
        self.nc.clear_and_free_semaphores(list(self.sems.allocated().values()))
        self.nc.all_engine_barrier()

    tile.TileContext._drain_and_barrier = _patched
    tile.TileContext._drain_split_installed = True


_install_drain_wait_split()


def build_gau_nc(S=S, SQ=S // 2, D=D, QK=QK, H=H, reps=1, use_dr=True,
                 has_vb=True, has_ob=True):
    DR = 2 if use_dr else 1
    PM = mybir.MatmulPerfMode.DoubleRow if use_dr else None
    """Build the SPMD Bass program for one core: full-seq (rotated) keys,
    queries = key rows [0:SQ)."""
    assert D % P == 0 and H % P == 0 and S % NB == 0 and SQ % NB == 0
    assert QK == P
    KD = D // P      # d chunks
    NSK = S // P     # key-side seq tiles
    SBK = S // NB    # key-side 512-blocks
    IB = SQ // NB    # query-side 512-blocks (i blocks)
    HC = H // P      # h 128-chunks
    HB = H // NB     # h 512-blocks
    JC = S // P      # j chunks
    DB = D // NB     # output d blocks
    ISUB = NB // P   # i subtiles per i-block
    W = S - P + SQ   # bias table width
    HQ = H // NB     # h quarters (512 each)
    D_HALF = min(512, D)  # bn_stats max free dim

    nc = bass.Bass("TRN2", target_bir_lowering=False, debug=False)

    # ---- DRAM I/O ----
    xk = nc.dram_tensor("xk", [S, D], F32, kind="ExternalInput").ap()
    whv = nc.dram_tensor("whv", [D, H], FP8, kind="ExternalInput").ap()
    whg = nc.dram_tensor("whg", [D, H], FP8, kind="ExternalInput").ap()
    wqk = nc.dram_tensor("wqk", [D, QK], FP8, kind="ExternalInput").ap()
    wo = nc.dram_tensor("wo", [H, D], FP8, kind="ExternalInput").ap()
    bqk = nc.dram_tensor("bqk", [QK], F32, kind="ExternalInput").ap()
    g0 = nc.dram_tensor("g0", [QK], F32, kind="ExternalInput").ap()  # gamma0/S
    b0 = nc.dram_tensor("b0", [QK], F32, kind="ExternalInput").ap()  # beta0/S
    g1 = nc.dram_tensor("g1", [QK], F32, kind="ExternalInput").ap()
    b1 = nc.dram_tensor("b1", [QK], F32, kind="ExternalInput").ap()
    bhv16 = nc.dram_tensor("bhv16", [H], BF16, kind="ExternalInput").ap()
    bhg = nc.dram_tensor("bhg", [H], F32, kind="ExternalInput").ap()
    bo16 = nc.dram_tensor("bo16", [D], BF16, kind="ExternalInput").ap()  # bo/descale
    bt = nc.dram_tensor("bt", [P, W], FP8, kind="ExternalInput").ap()
    out = nc.dram_tensor("out", [SQ, D], F32, kind="ExternalOutput").ap()

    # DRAM scratch (gate rows)
    gsc = nc.dram_tensor("gsc", [H, SQ], FP8, kind="Internal").ap()

    with tile.TileContext(nc) as tc:
        from contextlib import ExitStack

        with ExitStack() as const_ctx:
            # constants hoisted out of the reps loop: affine_select burns a
            # register per call and 2 identities x 25 reps exhausts the pool
            constp = const_ctx.enter_context(tc.tile_pool(name="const", bufs=1))
            ident = constp.tile([P, P], BF16)
            make_identity(nc, ident)
            ident8z = constp.tile([P, 2, P], FP8)
            nc.gpsimd.memset(ident8z[:, 1, :], 0.0)
            make_identity(nc, ident8z[:, 0, :])
            ones = constp.tile([1, P], BF16)
            nc.vector.memset(ones, 1.0)
            eps_sb = constp.tile([P, 1], F32)
            nc.vector.memset(eps_sb, LN_EPS)
            for _rep in range(reps):
              with ExitStack() as outer:
                # pools that live for the whole kernel
                singles = outer.enter_context(tc.tile_pool(name="singles", bufs=1))
                wpool = outer.enter_context(tc.tile_pool(name="wpool", bufs=1))
                qkpool = outer.enter_context(tc.tile_pool(name="qkpool", bufs=1))
                vsb = outer.enter_context(tc.tile_pool(name="vsb", bufs=1))
                ps_mm = outer.enter_context(
                    tc.tile_pool(name="ps_mm", bufs=4, space="PSUM"))

                # small parameter tiles
                bqk_sb = singles.tile([P, 1], F32)
                nc.sync.dma_start(bqk_sb, bqk.unsqueeze(1))
                g0_sb = singles.tile([P, 1], F32)
                nc.sync.dma_start(g0_sb, g0.unsqueeze(1))
                b0_sb = singles.tile([P, 1], F32)
                nc.sync.dma_start(b0_sb, b0.unsqueeze(1))
                g1_sb = singles.tile([P, 1], F32)
                nc.sync.dma_start(g1_sb, g1.unsqueeze(1))
                b1_sb = singles.tile([P, 1], F32)
                nc.sync.dma_start(b1_sb, b1.unsqueeze(1))
                bhg_sb = singles.tile([P, HC], F32)
                bo_row = singles.tile([1, D], BF16)
                # sim operands packed [P, 2, n] fp8 with a zero second chunk:
                # the q@k and ident@bt matmuls then run in DoubleRow at 0.5
                # cycles/row (half the PE cost); the zero chunk contributes 0.
                bt8z = qkpool.tile([P, 2, W], FP8)
                nc.vector.memset(bt8z[:, 1, :], 0.0)
                kT = qkpool.tile([P, 2, S], FP8)   # [qk-dim, (real,zero), s]
                nc.vector.memset(kT[:, 1, :], 0.0)
                qT = qkpool.tile([P, 2, SQ], FP8)  # [qk-dim, (real,zero), i]
                nc.vector.memset(qT[:, 1, :], 0.0)
                v_sb = vsb.tile([P, NSK, H], FP8)  # v rows, resident in SBUF

                L1 = outer.enter_context(ExitStack())
                big1 = L1.enter_context(tc.tile_pool(name="big1", bufs=1))
                # normed^T: query half (cols [0:SQ)) lives through gate-proj;
                # rest half frees right after the fused loop
                ntkq = big1.tile([P, KD, SQ], FP8, tag="ntkq")

                def nt(k0, k1, c0, c1):
                    if c0 >= SQ:
                        return ntkr[:, k0:k1, c0 - SQ:c1 - SQ]
                    assert c1 <= SQ
                    return ntkq[:, k0:k1, c0:c1]

                GT = 4  # seq tiles per transpose-evac group
                with ExitStack() as L2:
                    big2 = L2.enter_context(tc.tile_pool(name="big2", bufs=1))
                    work = L2.enter_context(tc.tile_pool(name="work", bufs=8))
                    nrmp = L2.enter_context(tc.tile_pool(name="nrmp", bufs=9))
                    stat = L2.enter_context(tc.tile_pool(name="stat", bufs=12))
                    ps_tr = L2.enter_context(
                        tc.tile_pool(name="ps_tr", bufs=4, space="PSUM"))

                    ntkr = big2.tile([P, KD, S - SQ], FP8, tag="ntkr")

                    wqk_sb = big2.tile([P, KD, QK], FP8, tag="wqk")
                    nc.gpsimd.dma_start(
                        wqk_sb, wqk.rearrange("(o p) q -> p o q", p=P))
                    bhv_row = big2.tile([1, H], BF16, tag="bhvr")
                    if has_vb:
                        nc.sync.dma_start(bhv_row, bhv16.unsqueeze(0))

                    whv_sb = wpool.tile([P, KD, H], FP8, tag="w")
                    nc.gpsimd.dma_start(
                        whv_sb, whv.rearrange("(o p) h -> p o h", p=P))

                    # ---- Fused: LayerNorm + transpose + qk-proj + v-proj ----
                    # one group = GT=4 seq tiles = one NB-wide column of ntk;
                    # norms for group g+1 are emitted BEFORE v-proj of group g
                    # so Act/DVE prepare the next group while PE drains this
                    # one.  (ln gamma/beta folded into weights host-side; evac
                    # is a pure copy on gpsimd)
                    nsub = D // D_HALF
                    assert NSK % GT == 0

                    def do_norms(g):
                        nrms = []
                        for tt in range(GT):
                            t = g * GT + tt
                            x_t = work.tile([P, D], F32, tag="xt")
                            nc.sync.dma_start(
                                x_t, xk[t * P:(t + 1) * P, :])
                            stats = stat.tile([P, nsub, 6], F32, tag="st")
                            for i in range(nsub):
                                nc.vector.bn_stats(
                                    out=stats[:, i, :],
                                    in_=x_t[:, i * D_HALF:(i + 1) * D_HALF])
                            mv = stat.tile([P, 2], F32, tag="mv")
                            nc.vector.bn_aggr(out=mv, in_=stats)
                            rstd = stat.tile([P, 1], F32, tag="rs")
                            nc.scalar.activation(
                                out=rstd, in_=mv[:, 1:2],
                                func=mybir.ActivationFunctionType.Sqrt,
                                bias=eps_sb, scale=1.0)
                            nc.vector.reciprocal(out=rstd, in_=rstd)
                            nm = stat.tile([P, 1], F32, tag="nm")
                            nc.vector.tensor_scalar(
                                out=nm, in0=mv[:, 0:1],
                                scalar1=rstd, scalar2=-1.0,
                                op0=mybir.AluOpType.mult,
                                op1=mybir.AluOpType.mult)
                            nrm = nrmp.tile([P, D], BF16, tag="nrm",
                                            name=f"nrm{tt}")
                            nc.vector.tensor_scalar(
                                out=nrm, in0=x_t,
                                scalar1=rstd, scalar2=nm,
                                op0=mybir.AluOpType.mult,
                                op1=mybir.AluOpType.add)
                            nrms.append(nrm)
                        return nrms

                    nrms_cur = do_norms(0)
                    # deferred parameter loads: issued after the first LN
                    # group's x tiles so the serialized DMA ramp-up serves
                    # the critical path first
                    nc.gpsimd.dma_start(bhg_sb, bhg.rearrange("(o p) -> p o", p=P))
                    if has_ob:
                        nc.gpsimd.dma_start(bo_row, bo16.unsqueeze(0))
                    nc.gpsimd.dma_start(bt8z[:, 0, :], bt)
                    whg_sb = None
                    for g in range(NSK // GT):
                        if g == NSK // GT - 1:
                            # prefetch the gate weights; transfer overlaps the
                            # last group's v-projection
                            whg_sb = wpool.tile([P, KD, H], FP8, tag="w")
                            nc.gpsimd.dma_start(
                                whg_sb, whg.rearrange("(o p) h -> p o h", p=P))
                        c0 = g * GT * P
                        nrms = nrms_cur
                        for k2 in range(0, KD, 2):
                            pst = ps_tr.tile([P, 2, GT, P], BF16, tag="pst")
                            for dk in range(2):
                                for tt in range(GT):
                                    nc.tensor.transpose(
                                        pst[:, dk, tt, :],
                                        nrms[tt][:, (k2 + dk) * P:
                                                  (k2 + dk + 1) * P], ident)
                            if k2 % 4 == 0:
                                nc.scalar.copy(
                                    out=nt(k2, k2 + 2, c0, c0 + GT * P),
                                    in_=pst)
                            else:
                                nc.vector.tensor_scalar_add(
                                    nt(k2, k2 + 2, c0, c0 + GT * P), pst, 0.0)

                        # k^T / q^T projection for this 512-column block
                        # (qT rows are a prefix-slice of the kT rows)
                        sb = g
                        ps = ps_mm.tile([P, NB], F32, tag="mm")
                        for k in range(0, KD, DR):
                            nc.tensor.matmul(
                                ps, wqk_sb[:, k:k + DR, :],
                                nt(k, k + DR, sb * NB, (sb + 1) * NB),
                                start=(k == 0), stop=(k == KD - DR),
                                perf_mode=PM)
                        tmp = work.tile([P, NB], BF16, tag="qtmp")
                        nc.scalar.activation(
                            out=tmp, in_=ps,
                            func=mybir.ActivationFunctionType.Silu,
                            bias=bqk_sb, scale=1.0)
                        nc.vector.tensor_scalar(
                            out=kT[:, 0, sb * NB:(sb + 1) * NB],
                            in0=tmp, scalar1=g1_sb, scalar2=b1_sb,
                            op0=mybir.AluOpType.mult,
                            op1=mybir.AluOpType.add)
                        if sb < IB:
                            nc.vector.tensor_scalar(
                                out=qT[:, 0, sb * NB:(sb + 1) * NB],
                                in0=tmp, scalar1=g0_sb, scalar2=b0_sb,
                                op0=mybir.AluOpType.mult,
                                op1=mybir.AluOpType.add)

                        if g + 1 < NSK // GT:
                            nrms_cur = do_norms(g + 1)

                        # v projection for this group's 4 seq tiles -> SBUF
                        for st in range(g * GT, (g + 1) * GT):
                            for hb in range(HB):
                                ps = ps_mm.tile([P, NB], F32, tag="mm")
                                if has_vb:
                                    nc.tensor.matmul(
                                        ps, ones,
                                        bhv_row[:, hb * NB:(hb + 1) * NB],
                                        start=True, stop=False)
                                for k in range(0, KD, DR):
                                    nc.tensor.matmul(
                                        ps, nt(k, k + DR, st * P, (st + 1) * P),
                                        whv_sb[:, k:k + DR, hb * NB:(hb + 1) * NB],
                                        start=(k == 0 and not has_vb),
                                        stop=(k == KD - DR),
                                        perf_mode=PM)
                                nc.scalar.activation(
                                    out=v_sb[:, st, hb * NB:(hb + 1) * NB],
                                    in_=ps,
                                    func=mybir.ActivationFunctionType.Silu)
                # L2 exits: ntkr / LN pools freed

                with ExitStack() as L3:
                    a2pool = L3.enter_context(tc.tile_pool(name="a2pool", bufs=2))
                    growp = L3.enter_context(tc.tile_pool(name="growp", bufs=6))
                    gopool = L3.enter_context(tc.tile_pool(name="gopool", bufs=1))
                    gpool = L3.enter_context(tc.tile_pool(name="gpool", bufs=2))
                    opool = L3.enter_context(tc.tile_pool(name="opool", bufs=3))
                    s1pool = L3.enter_context(
                        tc.tile_pool(name="s1pool", bufs=5))
                    ps_acc = L3.enter_context(
                        tc.tile_pool(name="ps_acc", bufs=ISUB, space="PSUM"))

                    def sim_block(ib):
                        # ib 0 runs inside the Act-saturated gate window:
                        # keep its relu^2 entirely on DVE.  For the rest,
                        # alternate the relu/square engines per j so neither
                        # engine sees a long burst.
                        attn2 = a2pool.tile([P, JC, NB], FP8, tag="attn2")
                        for j in range(JC):
                            ps = ps_mm.tile([P, NB], F32, tag="mm")
                            nc.tensor.matmul(
                                ps, kT[:, :, j * P:(j + 1) * P],
                                qT[:, :, ib * NB:(ib + 1) * NB],
                                start=True, stop=False, perf_mode=PM)
                            m0 = ib * NB - j * P + (S - P)
                            nc.tensor.matmul(
                                ps, ident8z, bt8z[:, :, m0:m0 + NB],
                                start=False, stop=True, perf_mode=PM)
                            s1 = s1pool.tile([P, NB], BF16, tag="s1")
                            nc.scalar.activation(
                                out=s1, in_=ps,
                                func=mybir.ActivationFunctionType.Relu)
                            nc.vector.tensor_mul(attn2[:, j, :], s1, s1)
                        return attn2

                    def attnv_quad(ib, hq, attn2, goT, gt_all):
                        for hh in range(NB // P):
                            hc = hq * (NB // P) + hh
                            pacc = ps_acc.tile([P, NB], F32, tag="pacc")
                            for j in range(0, JC, DR):
                                nc.tensor.matmul(
                                    pacc,
                                    v_sb[:, j:j + DR, hc * P:(hc + 1) * P],
                                    attn2[:, j:j + DR, :],
                                    start=(j == 0), stop=(j == JC - DR),
                                    perf_mode=PM)
                            nc.vector.tensor_mul(
                                goT[:, hc, :], pacc, gt_all[:, hh, :])

                    def load_gt(ib, hq):
                        gt_all = gpool.tile([P, NB // P, NB], FP8, tag="gt")
                        nc.sync.dma_start(
                            gt_all,
                            gsc[hq * NB:(hq + 1) * NB, ib * NB:(ib + 1) * NB]
                            .rearrange("(o p) i -> p o i", p=P))
                        return gt_all

                    def attnv_block(ib, attn2):
                        goT = gopool.tile([P, HC, NB], FP8, tag="goT")
                        for hq in range(HQ):
                            gt_all = load_gt(ib, hq)
                            attnv_quad(ib, hq, attn2, goT, gt_all)
                        return goT

                    def out_proj(ib, goT):
                        for isub in range(ISUB):
                            i0 = ib * NB + isub * P
                            xt = opool.tile([P, D], F32, tag="xres")
                            nc.sync.dma_start(xt, xk[i0:i0 + P, :])
                            ot = opool.tile([P, D], F32, tag="ot")
                            for db in range(DB):
                                ps = ps_mm.tile([P, NB], F32, tag="mm")
                                if has_ob:
                                    nc.tensor.matmul(
                                        ps, ones,
                                        bo_row[:, db * NB:(db + 1) * NB],
                                        start=True, stop=False)
                                for hc in range(0, HC, DR):
                                    nc.tensor.matmul(
                                        ps, goT[:, hc:hc + DR,
                                                isub * P:(isub + 1) * P],
                                        wo_sb[:, hc:hc + DR,
                                              db * NB:(db + 1) * NB],
                                        start=(hc == 0 and not has_ob),
                                        stop=(hc == HC - DR),
                                        perf_mode=PM)
                                nc.vector.scalar_tensor_tensor(
                                    out=ot[:, db * NB:(db + 1) * NB],
                                    in0=ps, scalar=ATTN_DESCALE,
                                    in1=xt[:, db * NB:(db + 1) * NB],
                                    op0=mybir.AluOpType.mult,
                                    op1=mybir.AluOpType.add)
                            nc.gpsimd.dma_start(out[i0:i0 + P, :], ot)

                    # sim for i-block 0, then gate projection interleaved with
                    # attn@v for i-block 0 (per 512-row hq quad): the gate
                    # silus (Act) overlap the attention matmuls (PE)
                    attn2_cur = sim_block(0)

                    goT_cur = gopool.tile([P, HC, NB], FP8, tag="goT")
                    grows = []
                    for hq in range(HQ):
                        for hh in range(NB // P):
                            hc = hq * (NB // P) + hh
                            grow = growp.tile([P, SQ], FP8, tag="grow")
                            for ibb in range(IB):
                                ps = ps_mm.tile([P, NB], F32, tag="mm")
                                for k in range(0, KD, DR):
                                    nc.tensor.matmul(
                                        ps, whg_sb[:, k:k + DR,
                                                   hc * P:(hc + 1) * P],
                                        nt(k, k + DR, ibb * NB, (ibb + 1) * NB),
                                        start=(k == 0), stop=(k == KD - DR),
                                        perf_mode=PM)
                                nc.scalar.activation(
                                    out=grow[:, ibb * NB:(ibb + 1) * NB],
                                    in_=ps,
                                    func=mybir.ActivationFunctionType.Silu,
                                    bias=bhg_sb[:, hc:hc + 1], scale=1.0)
                            nc.gpsimd.dma_start(
                                gsc[hc * P:(hc + 1) * P, :], grow)
                            grows.append(grow)
                        for hh in range(NB // P):
                            hc = hq * (NB // P) + hh
                            pacc = ps_acc.tile([P, NB], F32, tag="pacc")
                            for j in range(0, JC, DR):
                                nc.tensor.matmul(
                                    pacc,
                                    v_sb[:, j:j + DR, hc * P:(hc + 1) * P],
                                    attn2_cur[:, j:j + DR, :],
                                    start=(j == 0), stop=(j == JC - DR),
                                    perf_mode=PM)
                            nc.vector.tensor_mul(
                                goT_cur[:, hc, :], pacc, grows[hh][:, 0:NB])
                        grows = []

                    wo_sb = wpool.tile([P, HC, D], FP8, tag="w")
                    nc.gpsimd.dma_start(wo_sb, wo.rearrange("(o p) d -> p o d", p=P))

                    # ---------- attention + gating + out-proj ----------
                    for ib in range(IB):
                        # next i-block sim first: PE runs it after attn@v(ib),
                        # its relu^2 (DVE) overlaps the out-proj matmuls
                        if ib + 1 < IB:
                            attn2_cur = sim_block(ib + 1)
                        out_proj(ib, goT_cur)
                        if ib + 1 < IB:
                            goT_cur = attnv_block(ib + 1, attn2_cur)

    _split_excess_waits(nc)
    return nc


def _t5_bias_vec(rel_emb, S_, D_):
    """bv[r + S_-1] = bias for rel = k_pos - q_pos = r, scaled by sqrt(D)/S."""
    r = np.arange(-(S_ - 1), S_, dtype=np.int64)
    n = (-r).astype(np.int64)
    nb = NUM_BUCKETS // 2
    me = nb // 2
    ret = (n < 0).astype(np.int64) * nb
    na = np.abs(n)
    val_large = me + (
        np.log(np.maximum(na, 1).astype(np.float32) / me)
        / math.log(MAX_DIST / me) * (nb - me)).astype(np.int64)
    val_large = np.minimum(val_large, nb - 1)
    bucket = ret + np.where(na < me, na, val_large)
    return (rel_emb[bucket, 0].astype(np.float64)
            * (float(D_) ** 0.5) / float(S_)).astype(np.float32)


def make_core_inputs(inputs, S_=S, SQ_=None, D_=D, QK_=QK, H_=H,
                     n_cores=N_CORES):
    """Build per-core in_maps from the full (unsharded) input dict."""
    if SQ_ is None:
        SQ_ = S_ // 2
    bf = ml_dtypes.bfloat16
    f8 = ml_dtypes.float8_e4m3fn
    x = np.asarray(inputs["x"], np.float32)
    Wh = np.asarray(inputs["Wh"], np.float32)
    bh = np.asarray(inputs["bh"], np.float32)
    Wqk = np.asarray(inputs["Wqk"], np.float32)
    bqk_ = np.asarray(inputs["bqk"], np.float32)
    osg = np.asarray(inputs["os_gamma"], np.float32)
    osb = np.asarray(inputs["os_beta"], np.float32)
    Wo = np.asarray(inputs["Wo"], np.float32)
    bo_ = np.asarray(inputs["bo"], np.float32)
    rel_emb = np.asarray(inputs["rel_emb"], np.float32)
    lng_ = np.asarray(inputs["ln_g"], np.float32)
    lnb_ = np.asarray(inputs["ln_b"], np.float32)

    # fold LayerNorm gamma/beta into the projection weights:
    #   normed @ W + b  ==  z @ (g*W) + (b_ln @ W + b)   with z standardized
    Whv = Wh[:, :H_]
    Whg = Wh[:, H_:]
    whv_eff = lng_[:, None] * Whv
    whg_eff = lng_[:, None] * Whg
    wqk_eff = lng_[:, None] * Wqk
    bhv_eff = bh[:H_] + lnb_ @ Whv
    bhg_eff = bh[H_:] + lnb_ @ Whg
    bqk_eff = bqk_ + lnb_ @ Wqk

    bv = _t5_bias_vec(rel_emb, S_, D_)
    W_ = S_ - P + SQ_
    halves = S_ // SQ_

    shared = dict(
        whv=np.ascontiguousarray(whv_eff).astype(f8),
        whg=np.ascontiguousarray(whg_eff).astype(f8),
        wqk=np.ascontiguousarray(wqk_eff).astype(f8),
        wo=np.ascontiguousarray(Wo).astype(f8),
        bqk=bqk_eff.astype(np.float32),
        g0=(osg[0] / float(S_) * ATTN_PRESCALE).astype(np.float32),
        b0=(osb[0] / float(S_) * ATTN_PRESCALE).astype(np.float32),
        g1=osg[1].copy(), b1=osb[1].copy(),
        bhv16=bhv_eff.astype(bf),
        bhg=bhg_eff.astype(np.float32),
        bo16=(bo_ / ATTN_DESCALE).astype(bf),
    )

    # Toeplitz T5 bias table for the rotated key ordering:
    # bt[p, m] = bias(delta = p - m + S - P), with the wrap for rotated keys.
    pp = np.arange(P)[:, None]
    mm = np.arange(W_)[None, :]
    delta = pp - mm + (S_ - P)
    in_maps = []
    for c in range(n_cores):
        b = c // halves
        off = (c % halves) * SQ_
        d = np.where(delta < S_ - off, delta, delta - S_)
        btc = (bv[d + S_ - 1] * ATTN_PRESCALE).astype(f8)
        m = dict(shared)
        m["xk"] = np.ascontiguousarray(np.roll(x[b], -off, axis=0))
        m["bt"] = btc
        in_maps.append(m)
    flags = dict(has_vb=bool(np.any(bhv_eff)), has_ob=bool(np.any(bo_)))
    return in_maps, flags


def run_with_results(inputs, trace=False):
    in_maps, flags = make_core_inputs(inputs)
    key = (S, S // 2, D, QK, H, flags["has_vb"], flags["has_ob"])
    if key not in _NC_CACHE:
        _NC_CACHE[key] = build_gau_nc(S, S // 2, D, QK, H, **flags)
    nc = _NC_CACHE[key]
    res = run_bass_kernel_spmd(nc, in_maps, core_ids=list(range(N_CORES)),
                               trace=trace)
    SQ_ = S // 2
    halves = S // SQ_
    out = np.empty((B, S, D), np.float32)
    for c in range(N_CORES):
        b = c // halves
        off = (c % halves) * SQ_
        out[b, off:off + SQ_, :] = res.results[c]["out"]
    return out, res


def kernel(**inputs):
    return run_with_results(inputs, trace=False)[0]
